# revision 7
# baseline (speedup 1.0000x reference)
"""Trainium2 Bass kernel for MultiHeadSelfAttention (B=4, L=2048, H=1024, NH=16).

Sharding: 8 cores = 4 batches x 2 head-groups (8 heads each).
Per core: QKV projections (bf16 matmuls, contraction over H on partitions,
host-pretransposed inputs), S^T-layout attention (keys on partitions,
queries on free dim), exp on ScalarE (fused 1/8 scale), mask multiply on
VectorE (bf16 2x mode), AV + softmax-denominator via col-packed matmuls,
approximate-reciprocal division, output projection. Host sums the two
head-group partial outputs per batch and adds the output bias.
"""

import sys

try:
    import concourse.bass as bass  # noqa: F401
except ImportError:
    sys.path.insert(0, "/opt/trn_rl_repo")

import numpy as np
import ml_dtypes

import concourse.bass as bass
import concourse.mybir as mybir
import concourse.tile as tile
from concourse import bacc
from concourse import bass_utils

BF16 = mybir.dt.bfloat16
F32 = mybir.dt.float32

B, L, H = 4, 2048, 1024
NH, HD = 16, 64
NCORES = 8
HPC = NH // 2          # heads per core = 8
CPC = H // 2           # channels per core = 512
KAUG = 1152            # 1024 + 1 bias row, padded to 9*128
KC = KAUG // 128       # 9 contraction chunks
PAIRS = HPC // 2       # 4 head pairs per core
LT = L // 128          # 16 token tiles


def build_nc():
    nc = bacc.Bacc("TRN2", target_bir_lowering=False, debug=False,
                   num_devices=NCORES)

    xT = nc.dram_tensor("xT", [KAUG, L], BF16, kind="ExternalInput").ap()
    wqT = nc.dram_tensor("wqT", [KAUG, CPC], BF16, kind="ExternalInput").ap()
    wkT = nc.dram_tensor("wkT", [KAUG, CPC], BF16, kind="ExternalInput").ap()
    wvT = nc.dram_tensor("wvT", [KAUG, CPC], BF16, kind="ExternalInput").ap()
    woT = nc.dram_tensor("woT", [CPC, H], BF16, kind="ExternalInput").ap()
    maskT = nc.dram_tensor("maskT", [L, L], BF16, kind="ExternalInput").ap()
    out = nc.dram_tensor("out", [L, H], F32, kind="ExternalOutput").ap()

    with tile.TileContext(nc) as tc:
        mhsa_body(tc, xT, wqT, wkT, wvT, woT, maskT, out)
    nc.compile()
    return nc


def mhsa_body(tc, xT, wqT, wkT, wvT, woT, maskT, out):
    nc = tc.nc
    Exp = mybir.ActivationFunctionType.Exp
    mult = mybir.AluOpType.mult

    xT_r = xT.rearrange("(kc p) t -> p kc t", p=128)
    wq_r = wqT.rearrange("(kc p) c -> p kc c", p=128)
    wk_r = wkT.rearrange("(kc p) c -> p kc c", p=128)
    wv_r = wvT.rearrange("(kc p) c -> p kc c", p=128)
    wo_r = woT.rearrange("(kc p) c -> p kc c", p=128)
    mask_r = maskT.rearrange("(jt p) i -> p jt i", p=128)
    out_r = out.rearrange("(tt p) c -> p tt c", p=128)

    import contextlib
    ctx = contextlib.ExitStack()
    with ctx:
        consts = ctx.enter_context(tc.tile_pool(name="consts", bufs=1))
        wpool = ctx.enter_context(tc.tile_pool(name="weights", bufs=1))
        qkv_sb = ctx.enter_context(tc.tile_pool(name="qkv_sb", bufs=1))
        ao_pool = ctx.enter_context(tc.tile_pool(name="ao", bufs=1))

        wq_sb = wpool.tile([128, KC, CPC], BF16, tag="wq")
        wk_sb = wpool.tile([128, KC, CPC], BF16, tag="wk")
        wv_sb = wpool.tile([128, KC, CPC], BF16, tag="wv")
        nc.sync.dma_start(wq_sb[:], wq_r)
        nc.sync.dma_start(wk_sb[:], wk_r)
        nc.sync.dma_start(wv_sb[:], wv_r)

        qT_sb = qkv_sb.tile([128, PAIRS, L], BF16, tag="qT")
        kT_sb = qkv_sb.tile([128, PAIRS, L], BF16, tag="kT")
        v_sb = qkv_sb.tile([128, LT, 2 * CPC], BF16, tag="v")
        aoT_sb = ao_pool.tile([128, PAIRS, L], BF16, tag="aoT")

        # ---------------- Phase 1: QKV projections ----------------
        with tc.tile_pool(name="xpool", bufs=1) as xpool, \
             tc.tile_pool(name="qkv_ps", bufs=2, space="PSUM") as qkv_ps:
            x_sb = xpool.tile([128, KC, L], BF16, tag="x")
            for kc in range(KC):
                nc.sync.dma_start(x_sb[:, kc, :], xT_r[:, kc, :])

            # q^T, k^T: [CPC, L] laid out pair-major on partitions
            for mc in range(PAIRS):
                for nh in range(2):
                    for w_sb, dst in ((wq_sb, qT_sb), (wk_sb, kT_sb)):
                        ps = qkv_ps.tile([128, 1024], F32, tag="qk_ps")
                        for kc in range(KC):
                            for hf in range(2):
                                nc.tensor.matmul(
                                    ps[:, hf * 512:(hf + 1) * 512],
                                    w_sb[:, kc, mc * 128:(mc + 1) * 128],
                                    x_sb[:, kc,
                                         nh * 1024 + hf * 512:
                                         nh * 1024 + (hf + 1) * 512],
                                    start=(kc == 0), stop=(kc == KC - 1),
                                )
                        nc.scalar.copy(
                            dst[:, mc, nh * 1024:(nh + 1) * 1024], ps[:])
            # v: [L, CPC] natural layout, token tiles on partitions.
            # v_sb holds per head 64 v-columns then 64 ones-columns, so the
            # AV lhsT [128,128] computes numerator (rows 0-63) and softmax
            # denominator (rows 64-127) in one matmul.
            v_aug = v_sb[:].rearrange("p t (h two d) -> p t h two d",
                                      two=2, d=64)
            nc.any.memset(v_aug[:, :, :, 1, :], 1.0)
            for t in range(LT):
                ps = qkv_ps.tile([128, 512], F32, tag="v_ps")
                for kc in range(KC):
                    nc.tensor.matmul(
                        ps[:],
                        x_sb[:, kc, t * 128:(t + 1) * 128],
                        wv_sb[:, kc, :],
                        start=(kc == 0), stop=(kc == KC - 1),
                    )
                nc.scalar.copy(
                    v_aug[:, t, :, 0, :],
                    ps[:].rearrange("p (h d) -> p h d", d=64))

        # ---------------- Phase 2: attention per head ----------------
        with tc.tile_pool(name="mask", bufs=1) as mpool, \
             tc.tile_pool(name="ptiles", bufs=3) as ppool, \
             tc.tile_pool(name="rec", bufs=1) as rpool, \
             tc.tile_pool(name="s_ps", bufs=1, space="PSUM") as s_ps, \
             tc.tile_pool(name="av_ps", bufs=1, space="PSUM") as av_ps:

            mask_sb = mpool.tile([128, LT, L], BF16, tag="mask")
            for jt in range(LT):
                nc.sync.dma_start(mask_sb[:, jt, :], mask_r[:, jt, :])

            for h in range(HPC):
                p = h // 2
                rb = (h % 2) * 64
                # 4 accumulator banks, one per 512-wide query chunk:
                # partitions 0-63 = numerator (v rows), 64-127 = denominator
                nd = [av_ps.tile([128, 512], F32, tag=f"nd{ic}",
                                 name=f"nd{h}_{ic}") for ic in range(4)]
                for j in range(LT):
                    s = s_ps.tile([128, 2048], F32, tag="s")
                    for ic in range(4):
                        nc.tensor.matmul(
                            s[:, ic * 512:(ic + 1) * 512],
                            kT_sb[rb:rb + 64, p, j * 128:(j + 1) * 128],
                            qT_sb[rb:rb + 64, p, ic * 512:(ic + 1) * 512],
                            start=True, stop=True,
                        )
                    pm = ppool.tile([128, 2048], BF16, tag="pm")
                    nc.scalar.activation(pm[:], s[:], Exp, scale=0.125)
                    nc.vector.tensor_tensor(
                        pm[:], pm[:], mask_sb[:, j, :], mult)
                    for ic in range(4):
                        nc.tensor.matmul(
                            nd[ic][:],
                            v_sb[:, j, h * 128:(h + 1) * 128],
                            pm[:, ic * 512:(ic + 1) * 512],
                            start=(j == 0), stop=(j == LT - 1),
                        )
                for ic in range(4):
                    den_cp = rpool.tile([64, 512], F32, tag="den_cp",
                                        name=f"dcp{h}_{ic}")
                    nc.vector.tensor_copy(den_cp[:], nd[ic][64:128, :])
                    rec = rpool.tile([64, 512], F32, tag="rec",
                                     name=f"rec{h}_{ic}")
                    nc.vector.reciprocal_approx_fast(rec[:], den_cp[:])
                    nc.vector.tensor_tensor(
                        aoT_sb[rb:rb + 64, p, ic * 512:(ic + 1) * 512],
                        nd[ic][0:64, :],
                        rec[:],
                        mult)

        # ---------------- Phase 3: output projection ----------------
        with tc.tile_pool(name="o_ps", bufs=2, space="PSUM") as o_ps, \
             tc.tile_pool(name="o_sb", bufs=3) as o_sbp, \
             tc.tile_pool(name="wo_pool", bufs=1) as wo_pool:
            wo_sb = wo_pool.tile([128, 4, H], BF16, tag="wo")
            nc.sync.dma_start(wo_sb[:], wo_r)
            for t in range(LT):
                ps = o_ps.tile([128, 1024], F32, tag="o")
                for kc in range(4):
                    for hf in range(2):
                        nc.tensor.matmul(
                            ps[:, hf * 512:(hf + 1) * 512],
                            aoT_sb[:, kc, t * 128:(t + 1) * 128],
                            wo_sb[:, kc, hf * 512:(hf + 1) * 512],
                            start=(kc == 0), stop=(kc == 3),
                        )
                o_sb = o_sbp.tile([128, 1024], F32, tag="o_sb")
                nc.scalar.copy(o_sb[:], ps[:])
                nc.sync.dma_start(out_r[:, t, :], o_sb[:])


_NC_CACHE = None


def get_nc():
    global _NC_CACHE
    if _NC_CACHE is None:
        _NC_CACHE = build_nc()
    return _NC_CACHE


def make_in_maps(x, attn_mask, Wq, bq, Wk, bk, Wv, bv, Wo, bo):
    bf = ml_dtypes.bfloat16
    x = np.asarray(x, np.float32)
    attn_mask = np.asarray(attn_mask)
    in_maps = []
    for core in range(NCORES):
        b, pg = divmod(core, 2)
        cs = slice(pg * CPC, (pg + 1) * CPC)
        xT = np.zeros((KAUG, L), bf)
        xT[:H] = x[b].T.astype(bf)
        xT[H] = 1.0
        m = {"xT": xT}
        for name, W, bias in (("wqT", Wq, bq), ("wkT", Wk, bk),
                              ("wvT", Wv, bv)):
            wT = np.zeros((KAUG, CPC), bf)
            wT[:H] = np.asarray(W, np.float32)[cs, :].T.astype(bf)
            wT[H] = np.asarray(bias, np.float32)[cs].astype(bf)
            m[name] = wT
        m["woT"] = np.ascontiguousarray(
            np.asarray(Wo, np.float32)[:, cs].T).astype(bf)
        m["maskT"] = np.ascontiguousarray(
            attn_mask[b, 0].T).astype(bf)
        in_maps.append(m)
    return in_maps


def gather(results, bo):
    bo = np.asarray(bo, np.float32)
    out = np.empty((B, L, H), np.float32)
    for b in range(B):
        out[b] = results[2 * b]["out"] + results[2 * b + 1]["out"] + bo
    return out


def kernel(x, attn_mask, Wq, bq, Wk, bk, Wv, bv, Wo, bo):
    nc = get_nc()
    in_maps = make_in_maps(x, attn_mask, Wq, bq, Wk, bk, Wv, bv, Wo, bo)
    res = bass_utils.run_bass_kernel_spmd(nc, in_maps,
                                          core_ids=list(range(NCORES)))
    return gather(res.results, bo)


# revision 8
# speedup vs baseline: 1.2880x; 1.2880x over previous
"""Trainium2 Bass kernel for MultiHeadSelfAttention (B=4, L=2048, H=1024, NH=16).

Sharding: 8 cores = 4 batches x 2 head-groups (8 heads each).
Per core: QKV projections (bf16 matmuls, contraction over H on partitions,
host-pretransposed inputs), S^T-layout attention (keys on partitions,
queries on free dim), exp on ScalarE (fused 1/8 scale), mask multiply on
VectorE (bf16 2x mode), AV + softmax-denominator via col-packed matmuls,
approximate-reciprocal division, output projection. Host sums the two
head-group partial outputs per batch and adds the output bias.
"""

import sys

try:
    import concourse.bass as bass  # noqa: F401
except ImportError:
    sys.path.insert(0, "/opt/trn_rl_repo")

import numpy as np
import ml_dtypes

import concourse.bass as bass
import concourse.mybir as mybir
import concourse.tile as tile
from concourse import bacc
from concourse import bass_utils

BF16 = mybir.dt.bfloat16
F32 = mybir.dt.float32

B, L, H = 4, 2048, 1024
NH, HD = 16, 64
NCORES = 8
HPC = NH // 2          # heads per core = 8
CPC = H // 2           # channels per core = 512
KAUG = 1152            # 1024 + 1 bias row, padded to 9*128
KC = KAUG // 128       # 9 contraction chunks
PAIRS = HPC // 2       # 4 head pairs per core
LT = L // 128          # 16 token tiles


def build_nc():
    nc = bacc.Bacc("TRN2", target_bir_lowering=False, debug=False,
                   num_devices=NCORES)

    xT = nc.dram_tensor("xT", [KAUG, L], BF16, kind="ExternalInput").ap()
    wqT = nc.dram_tensor("wqT", [KAUG, CPC], BF16, kind="ExternalInput").ap()
    wkT = nc.dram_tensor("wkT", [KAUG, CPC], BF16, kind="ExternalInput").ap()
    wvT = nc.dram_tensor("wvT", [KAUG, CPC], BF16, kind="ExternalInput").ap()
    woT = nc.dram_tensor("woT", [CPC, H], BF16, kind="ExternalInput").ap()
    maskT = nc.dram_tensor("maskT", [L, L], BF16, kind="ExternalInput").ap()
    out = nc.dram_tensor("out", [L, H], F32, kind="ExternalOutput").ap()

    with tile.TileContext(nc) as tc:
        mhsa_body(tc, xT, wqT, wkT, wvT, woT, maskT, out)
    nc.compile()
    return nc


def mhsa_body(tc, xT, wqT, wkT, wvT, woT, maskT, out):
    nc = tc.nc
    Exp = mybir.ActivationFunctionType.Exp
    mult = mybir.AluOpType.mult

    xT_r = xT.rearrange("(kc p) t -> p kc t", p=128)
    wq_r = wqT.rearrange("(kc p) c -> p kc c", p=128)
    wk_r = wkT.rearrange("(kc p) c -> p kc c", p=128)
    wv_r = wvT.rearrange("(kc p) c -> p kc c", p=128)
    wo_r = woT.rearrange("(kc p) c -> p kc c", p=128)
    mask_r = maskT.rearrange("(jt p) i -> p jt i", p=128)
    out_r = out.rearrange("(tt p) c -> p tt c", p=128)

    import contextlib
    ctx = contextlib.ExitStack()
    with ctx:
        consts = ctx.enter_context(tc.tile_pool(name="consts", bufs=1))
        wpool = ctx.enter_context(tc.tile_pool(name="weights", bufs=1))
        qkv_sb = ctx.enter_context(tc.tile_pool(name="qkv_sb", bufs=1))
        ao_pool = ctx.enter_context(tc.tile_pool(name="ao", bufs=1))

        wq_sb = wpool.tile([128, KC, CPC], BF16, tag="wq")
        wk_sb = wpool.tile([128, KC, CPC], BF16, tag="wk")
        wv_sb = wpool.tile([128, KC, CPC], BF16, tag="wv")
        nc.sync.dma_start(wq_sb[:], wq_r)
        nc.sync.dma_start(wk_sb[:], wk_r)
        nc.sync.dma_start(wv_sb[:], wv_r)

        qT_sb = qkv_sb.tile([128, PAIRS, L], BF16, tag="qT")
        kT_sb = qkv_sb.tile([128, PAIRS, L], BF16, tag="kT")
        v_sb = qkv_sb.tile([128, LT, 2 * CPC], BF16, tag="v")
        aoT_sb = ao_pool.tile([128, PAIRS, L], BF16, tag="aoT")

        # ---------------- Phase 1: QKV projections ----------------
        with tc.tile_pool(name="xpool", bufs=1) as xpool, \
             tc.tile_pool(name="qkv_ps", bufs=2, space="PSUM") as qkv_ps:
            x_sb = xpool.tile([128, KC, L], BF16, tag="x")
            for kc in range(KC):
                nc.sync.dma_start(x_sb[:, kc, :], xT_r[:, kc, :])

            # q^T, k^T: [CPC, L] laid out pair-major on partitions
            for mc in range(PAIRS):
                for nh in range(2):
                    for w_sb, dst in ((wq_sb, qT_sb), (wk_sb, kT_sb)):
                        ps = qkv_ps.tile([128, 1024], F32, tag="qk_ps")
                        for kc in range(KC):
                            for hf in range(2):
                                nc.tensor.matmul(
                                    ps[:, hf * 512:(hf + 1) * 512],
                                    w_sb[:, kc, mc * 128:(mc + 1) * 128],
                                    x_sb[:, kc,
                                         nh * 1024 + hf * 512:
                                         nh * 1024 + (hf + 1) * 512],
                                    start=(kc == 0), stop=(kc == KC - 1),
                                )
                        nc.scalar.copy(
                            dst[:, mc, nh * 1024:(nh + 1) * 1024], ps[:])
            # v: [L, CPC] natural layout, token tiles on partitions.
            # v_sb holds per head 64 v-columns then 64 ones-columns, so the
            # AV lhsT [128,128] computes numerator (rows 0-63) and softmax
            # denominator (rows 64-127) in one matmul.
            v_aug = v_sb[:].rearrange("p t (h two d) -> p t h two d",
                                      two=2, d=64)
            nc.any.memset(v_aug[:, :, :, 1, :], 1.0)
            for t in range(LT):
                ps = qkv_ps.tile([128, 512], F32, tag="v_ps")
                for kc in range(KC):
                    nc.tensor.matmul(
                        ps[:],
                        x_sb[:, kc, t * 128:(t + 1) * 128],
                        wv_sb[:, kc, :],
                        start=(kc == 0), stop=(kc == KC - 1),
                    )
                nc.scalar.copy(
                    v_aug[:, t, :, 0, :],
                    ps[:].rearrange("p (h d) -> p h d", d=64))

        # ---------------- Phase 2: attention per head ----------------
        with tc.tile_pool(name="mask", bufs=1) as mpool, \
             tc.tile_pool(name="ptiles", bufs=3) as ppool, \
             tc.tile_pool(name="rec", bufs=1) as rpool, \
             tc.tile_pool(name="s_ps", bufs=2, space="PSUM") as s_ps, \
             tc.tile_pool(name="av_ps", bufs=1, space="PSUM") as av_ps:

            mask_sb = mpool.tile([128, LT, L], BF16, tag="mask")
            for jt in range(LT):
                nc.sync.dma_start(mask_sb[:, jt, :], mask_r[:, jt, :])

            for h in range(HPC):
                p = h // 2
                rb = (h % 2) * 64
                # 4 accumulator banks, one per 512-wide query chunk:
                # partitions 0-63 = numerator (v rows), 64-127 = denominator
                nd = [av_ps.tile([128, 512], F32, tag=f"nd{ic}",
                                 name=f"nd{h}_{ic}") for ic in range(4)]
                for j in range(LT):
                    for ih in range(2):
                        s = s_ps.tile([128, 1024], F32, tag="s")
                        for ic2 in range(2):
                            ic = ih * 2 + ic2
                            nc.tensor.matmul(
                                s[:, ic2 * 512:(ic2 + 1) * 512],
                                kT_sb[rb:rb + 64, p, j * 128:(j + 1) * 128],
                                qT_sb[rb:rb + 64, p, ic * 512:(ic + 1) * 512],
                                start=True, stop=True,
                            )
                        pm = ppool.tile([128, 1024], BF16, tag="pm")
                        nc.scalar.activation(pm[:], s[:], Exp, scale=0.125)
                        nc.vector.tensor_tensor(
                            pm[:], pm[:],
                            mask_sb[:, j, ih * 1024:(ih + 1) * 1024], mult)
                        for ic2 in range(2):
                            ic = ih * 2 + ic2
                            nc.tensor.matmul(
                                nd[ic][:],
                                v_sb[:, j, h * 128:(h + 1) * 128],
                                pm[:, ic2 * 512:(ic2 + 1) * 512],
                                start=(j == 0), stop=(j == LT - 1),
                            )
                for ic in range(4):
                    den_cp = rpool.tile([64, 512], F32, tag="den_cp",
                                        name=f"dcp{h}_{ic}")
                    nc.vector.tensor_copy(den_cp[:], nd[ic][64:128, :])
                    rec = rpool.tile([64, 512], F32, tag="rec",
                                     name=f"rec{h}_{ic}")
                    nc.vector.reciprocal_approx_fast(rec[:], den_cp[:])
                    nc.vector.tensor_tensor(
                        aoT_sb[rb:rb + 64, p, ic * 512:(ic + 1) * 512],
                        nd[ic][0:64, :],
                        rec[:],
                        mult)

        # ---------------- Phase 3: output projection ----------------
        with tc.tile_pool(name="o_ps", bufs=2, space="PSUM") as o_ps, \
             tc.tile_pool(name="o_sb", bufs=3) as o_sbp, \
             tc.tile_pool(name="wo_pool", bufs=1) as wo_pool:
            wo_sb = wo_pool.tile([128, 4, H], BF16, tag="wo")
            nc.sync.dma_start(wo_sb[:], wo_r)
            for t in range(LT):
                ps = o_ps.tile([128, 1024], F32, tag="o")
                for kc in range(4):
                    for hf in range(2):
                        nc.tensor.matmul(
                            ps[:, hf * 512:(hf + 1) * 512],
                            aoT_sb[:, kc, t * 128:(t + 1) * 128],
                            wo_sb[:, kc, hf * 512:(hf + 1) * 512],
                            start=(kc == 0), stop=(kc == 3),
                        )
                o_sb = o_sbp.tile([128, 1024], F32, tag="o_sb")
                nc.scalar.copy(o_sb[:], ps[:])
                nc.sync.dma_start(out_r[:, t, :], o_sb[:])


_NC_CACHE = None


def get_nc():
    global _NC_CACHE
    if _NC_CACHE is None:
        _NC_CACHE = build_nc()
    return _NC_CACHE


def make_in_maps(x, attn_mask, Wq, bq, Wk, bk, Wv, bv, Wo, bo):
    bf = ml_dtypes.bfloat16
    x = np.asarray(x, np.float32)
    attn_mask = np.asarray(attn_mask)
    in_maps = []
    for core in range(NCORES):
        b, pg = divmod(core, 2)
        cs = slice(pg * CPC, (pg + 1) * CPC)
        xT = np.zeros((KAUG, L), bf)
        xT[:H] = x[b].T.astype(bf)
        xT[H] = 1.0
        m = {"xT": xT}
        for name, W, bias in (("wqT", Wq, bq), ("wkT", Wk, bk),
                              ("wvT", Wv, bv)):
            wT = np.zeros((KAUG, CPC), bf)
            wT[:H] = np.asarray(W, np.float32)[cs, :].T.astype(bf)
            wT[H] = np.asarray(bias, np.float32)[cs].astype(bf)
            m[name] = wT
        m["woT"] = np.ascontiguousarray(
            np.asarray(Wo, np.float32)[:, cs].T).astype(bf)
        m["maskT"] = np.ascontiguousarray(
            attn_mask[b, 0].T).astype(bf)
        in_maps.append(m)
    return in_maps


def gather(results, bo):
    bo = np.asarray(bo, np.float32)
    out = np.empty((B, L, H), np.float32)
    for b in range(B):
        out[b] = results[2 * b]["out"] + results[2 * b + 1]["out"] + bo
    return out


def kernel(x, attn_mask, Wq, bq, Wk, bk, Wv, bv, Wo, bo):
    nc = get_nc()
    in_maps = make_in_maps(x, attn_mask, Wq, bq, Wk, bk, Wv, bv, Wo, bo)
    res = bass_utils.run_bass_kernel_spmd(nc, in_maps,
                                          core_ids=list(range(NCORES)))
    return gather(res.results, bo)
